# revision 22
# baseline (speedup 1.0000x reference)
"""Trainium2 Bass kernel for nn_Attention_78280073937702.

Dense transformer attention block (prefill, B=1, S=2048, H=4096, 32 heads,
head_dim=128, fp32) sharded tensor-parallel over heads across 8 NeuronCores
(4 heads per core), with an AllToAll reshard so o_proj is sequence-sharded.

Host side pre-tiles and pre-casts everything to bf16 in partition-major
contiguous layouts, so every DMA moves multi-KB contiguous runs per
partition (the DMA system is packet-count bound, not byte bound):
  hid_t  [32][128][2048]  hidden^T as per-h-tile [p][s] blocks
  wqk_t  [8][128][32][128] W_pack^T q/k cols per pt (q0,k0,q1,k1,...)
  wtv_t  [128][32][512]    W_pack^T v cols, partition-major
  wo_pre_t  [8][128][24][512]  W_o^T rows for heads 0-2, per o-chunk
  wo_post_t [8][128][8][512]   W_o^T rows for head 3, per o-chunk
  cos_t/sin_t [128][2048]  RoPE tables (from position_ids)

Device per core:
  1. V proj -> vkeep in SBUF (no DRAM roundtrip)
  2. per head: Q proj, K proj (PSUM fp32 -> bf16), RoPE (PE half-swap matmul
     + DVE mul/add, all-bf16 operands), kept in SBUF
  3. causal attention per head (S^T tiles -> exp bf16 -> mask -> esum bf16
     tree -> Z via ones-matmul colsum -> U^T = V E in PSUM -> attn^T bf16).
     Attention is emitted as a queue of small steps pumped one at a time
     between projection (and o_proj) matmuls, so the PE never stalls on the
     scalar/vector exp chain and the HAM clock gate never re-throttles.
  4. per head AllToAll resharding attn^T from head-sharded to seq-sharded
  5. o_proj: out[s, o] natural layout, stationary = attnT tiles, moving =
     streamed W_o^T -> out_s [256, 4096] f32.  Head 3's attention steps pump
     into the pre-phase chunks so the last AllToAll overlaps the pre-phase;
     head 3's weights fully preload during the pre-phase so the post-phase
     does no DMA at all.

Host concatenates the 8 out_s slices along s.
"""

import os
import sys
from collections import deque
from contextlib import ExitStack

import numpy as np
import ml_dtypes

for _p in ("/opt/trn_rl_repo", os.path.expanduser("~/.axon_site/_ro/trn_rl_repo")):
    if os.path.isdir(_p) and _p not in sys.path:
        sys.path.insert(0, _p)

import concourse.bacc as bacc  # noqa: E402
import concourse.bass as bass  # noqa: E402
import concourse.mybir as mybir  # noqa: E402
import concourse.tile as tile  # noqa: E402
from concourse.alu_op_type import AluOpType  # noqa: E402
from concourse.bass_utils import run_bass_kernel_spmd  # noqa: E402

F32 = mybir.dt.float32
BF16 = mybir.dt.bfloat16
EXPF = mybir.ActivationFunctionType.Exp

N_CORES = 8
S = 2048
H = 4096
D = 128
P = 128
N_HEADS = 32
NH_LOC = N_HEADS // N_CORES  # 4 heads per core
HT = H // P  # 32 h-tiles
ST = S // P  # 16 s-tiles
SL = 512  # s-slice width for matmul free dim
NSL = S // SL  # 4
VC = NH_LOC * D  # 512 local v columns
SC = S // N_CORES  # 256 seq cols per core after reshard
NORM = 1.0 / float(np.sqrt(D))


def build_nc():
    nc = bacc.Bacc("TRN2", target_bir_lowering=False, num_devices=N_CORES)

    hid_d = nc.dram_tensor("hid_t", [NSL, HT, P, SL], BF16,
                           kind="ExternalInput")
    wqk_d = nc.dram_tensor("wqk_t", [2 * NH_LOC, P, HT, P], BF16,
                           kind="ExternalInput")
    wtv_d = nc.dram_tensor("wtv_t", [P, HT, VC], BF16, kind="ExternalInput")
    wopre_d = nc.dram_tensor("wo_pre_t", [8, P, 24, SL], BF16,
                             kind="ExternalInput")
    wopost_d = nc.dram_tensor("wo_post_t", [8, P, 8, SL], BF16,
                              kind="ExternalInput")
    cos_d = nc.dram_tensor("cos_t", [D, S], BF16, kind="ExternalInput")
    sin_d = nc.dram_tensor("sin_t", [D, S], BF16, kind="ExternalInput")
    out_d = nc.dram_tensor("out_s", [SC, H], F32, kind="ExternalOutput")

    with tile.TileContext(nc) as tc, ExitStack() as ctx:
        dram = ctx.enter_context(tc.tile_pool(name="dram", bufs=1, space="DRAM"))
        attn_loc = [
            dram.tile([N_CORES, P, SC], BF16, name=f"aloc{h}")
            for h in range(NH_LOC)
        ]
        attn_recv = [
            dram.tile([N_CORES, P, SC], BF16, name=f"arecv{h}")
            for h in range(NH_LOC)
        ]

        # ---------------- constants ----------------
        consts = ctx.enter_context(tc.tile_pool(name="consts", bufs=1))
        tri01_b = consts.tile([P, P], BF16)
        ones_b = consts.tile([P, P], BF16)
        p_swap_b = consts.tile([P, P], BF16)
        with tc.tile_pool(name="cscratch", bufs=1) as cs:
            ones_t = cs.tile([P, P], F32)
            nc.gpsimd.memset(ones_t, 1.0)
            # upper-triangular-with-diag keep-mask [k, q]: keep q >= k
            tri01 = cs.tile([P, P], F32)
            nc.gpsimd.affine_select(
                out=tri01, in_=ones_t, compare_op=AluOpType.is_ge,
                fill=0.0, base=0, channel_multiplier=-1, pattern=[[1, P]],
            )
            nc.vector.tensor_copy(tri01_b, tri01)
            nc.vector.tensor_copy(ones_b, ones_t)
            # signed half-swap lhsT: [i, i+64] = +1 (i<64), [i, i-64] = -1
            neg_t = cs.tile([P, P], F32)
            nc.gpsimd.memset(neg_t, -1.0)
            sw_pos = cs.tile([P, P], F32)
            nc.gpsimd.affine_select(
                out=sw_pos, in_=ones_t, compare_op=AluOpType.is_equal,
                fill=0.0, base=-64, channel_multiplier=-1, pattern=[[1, P]],
            )
            sw_neg = cs.tile([P, P], F32)
            nc.gpsimd.affine_select(
                out=sw_neg, in_=neg_t, compare_op=AluOpType.is_equal,
                fill=0.0, base=64, channel_multiplier=-1, pattern=[[1, P]],
            )
            p_swap = cs.tile([P, P], F32)
            nc.vector.tensor_add(p_swap, sw_pos, sw_neg)
            nc.vector.tensor_copy(p_swap_b, p_swap)

        # long-lived SBUF state (created before the hidden-resident pools)
        qk_keep = ctx.enter_context(tc.tile_pool(name="qkkeep", bufs=2))
        vk_pool = ctx.enter_context(tc.tile_pool(name="vkeep", bufs=1))
        e_pool = ctx.enter_context(tc.tile_pool(name="epool", bufs=4))
        z_pool = ctx.enter_context(tc.tile_pool(name="zpool", bufs=2))
        zr_pool = ctx.enter_context(tc.tile_pool(name="zrpool", bufs=1))
        att_pool = ctx.enter_context(tc.tile_pool(name="attst", bufs=1))
        at_pool = ctx.enter_context(tc.tile_pool(name="atT", bufs=1))
        st_ps_pool = ctx.enter_context(
            tc.tile_pool(name="stpsum", bufs=2, space="PSUM"))
        u_ps_pool = ctx.enter_context(
            tc.tile_pool(name="upsum", bufs=2, space="PSUM"))

        # V stays in SBUF: [p(s-within-tile)][st][c] with c = hh*128 + d
        vkeep = vk_pool.tile([P, ST, VC], BF16)
        # o_proj stationary for head 0 (loaded right after its AllToAll);
        # heads 1-3 tiles live in the o_proj scope
        attnT_h = [at_pool.tile([P, 8, SC], BF16, name="attnT0"),
                   None, None, None]

        def load_attnT(hh, eng):
            for g in range(N_CORES):
                eng.dma_start(attnT_h[hh][:, g, :], attn_recv[hh][g])

        def on_collective(h):
            # h0: tile exists from the start; h3: its tile exists by the
            # time this fires (pumped inside the o_proj scope); h1/h2 are
            # loaded at o_proj start instead
            if h in (0, 3):
                load_attnT(h, nc.gpsimd)

        # ---------- attention step queue (pumped between matmuls) ----------
        pending = deque()

        def pump(n=1):
            for _ in range(n):
                if pending:
                    pending.popleft()()

        def pump_all():
            while pending:
                pending.popleft()()

        def attn_block(h, qk, j):
            """Queue the emission steps for attention block (h, j)."""
            nkt = 4 * j + 4
            blk = {}

            def s_step(i):
                def f():
                    r = i - 4 * j
                    off = max(0, r) * P
                    if i == 0:
                        blk["esum"] = z_pool.tile([P, SL], BF16, tag="es",
                                                  name="esum")
                        blk["et"] = []
                    et = e_pool.tile([P, SL], BF16, tag="e", name="et")
                    sp = st_ps_pool.tile([P, SL], F32, tag="st", name="sp")
                    nc.tensor.matmul(
                        sp[:, off:],
                        qk[:, 1, i * P:(i + 1) * P],
                        qk[:, 0, j * SL + off:(j + 1) * SL],
                        start=True, stop=True,
                    )
                    nc.scalar.activation(
                        et[:, off:], sp[:, off:], EXPF, scale=NORM)
                    if r >= 0:
                        nc.vector.tensor_tensor(
                            et[:, off:off + P], et[:, off:off + P],
                            tri01_b, AluOpType.mult)
                    if i == 0:
                        nc.vector.tensor_copy(blk["esum"], et)
                    else:
                        nc.vector.tensor_tensor(
                            blk["esum"][:, off:], blk["esum"][:, off:],
                            et[:, off:], AluOpType.add)
                    blk["et"].append(et)
                return f

            def u_step(ui):
                def f():
                    if ui == 0:
                        blk["u"] = u_ps_pool.tile([P, SL], F32, tag="u",
                                                  name="u")
                    uoff = max(0, ui - 4 * j) * P
                    nc.tensor.matmul(
                        blk["u"][:, uoff:],
                        vkeep[:, ui, h * P:(h + 1) * P],
                        blk["et"][ui][:, uoff:],
                        start=(ui == 0), stop=(ui == nkt - 1),
                    )
                return f

            def flush_step():
                def f():
                    # z colsum shares the u psum slots
                    zb = u_ps_pool.tile([P, SL], F32, tag="u", name="zb")
                    nc.tensor.matmul(zb, ones_b, blk["esum"],
                                     start=True, stop=True)
                    zr = zr_pool.tile([P, SL], F32, tag="zr", name="zr")
                    nc.vector.reciprocal(zr, zb)
                    att = att_pool.tile([P, SL], BF16, tag="a", name="att")
                    nc.vector.tensor_tensor(att, blk["u"], zr, AluOpType.mult)
                    # stores + collective ride the idle gpsimd ring
                    nc.gpsimd.dma_start(attn_loc[h][2 * j], att[:, :SC])
                    nc.gpsimd.dma_start(attn_loc[h][2 * j + 1], att[:, SC:])
                    if j == NSL - 1:
                        nc.gpsimd.collective_compute(
                            "AllToAll", AluOpType.bypass,
                            replica_groups=[list(range(N_CORES))],
                            ins=[attn_loc[h][:].opt()],
                            outs=[attn_recv[h][:].opt()],
                        )
                        on_collective(h)
                return f

            for i in range(nkt):
                pending.append(s_step(i))
                if i >= 3:
                    pending.append(u_step(i - 3))
            for ui in range(nkt - 3, nkt):
                pending.append(u_step(ui))
            pending.append(flush_step())

        with ExitStack() as ab:  # projection phase
            hidT_pool = ab.enter_context(tc.tile_pool(name="hidT", bufs=1))
            # one tile per (s-slice, h-tile): [p][512], each a fully
            # contiguous 0.25MB DMA; many small DMAs keep many HW queues
            # busy (per-queue bandwidth is only ~22GB/s)
            hidT_t = [[hidT_pool.tile([P, SL], BF16, name=f"h{c}t{t}")
                       for t in range(HT)] for c in range(NSL)]

            def hid_mv(sl, ht):  # moving [P, SL] for s-slice sl, h-tile ht
                return hidT_t[sl][ht]

            def hid_st(st, ht):  # stationary [P, P] for s-tile st, h-tile ht
                return hidT_t[st // 4][ht][:, (st % 4) * P:(st % 4 + 1) * P]

            # ---------------- phase A: V projection ----------------
            with ExitStack() as vblk:
                wtv_pool = vblk.enter_context(tc.tile_pool(name="wtv", bufs=1))
                vps_pool = vblk.enter_context(
                    tc.tile_pool(name="vpsum", bufs=4, space="PSUM"))
                wtv_p = [wtv_pool.tile([P, 4, VC], BF16, name=f"wtv{g}")
                         for g in range(8)]
                # interleave weight/activation pieces in consumption
                # order, alternating rings (each engine ring feeds only ~8
                # HW queues at ~22GB/s each, so one ring tops out ~176GB/s)
                rr = [nc.sync, nc.scalar]
                for g in range(8):
                    rr[g % 2].dma_start(wtv_p[g], wtv_d[:, 4 * g:4 * g + 4, :])
                    for t in range(4 * g, 4 * g + 4):
                        rr[(t + 1) % 2].dma_start(hidT_t[0][t], hid_d[0, t])
                for c in range(1, NSL):
                    for t in range(HT):
                        rr[t % 2].dma_start(hidT_t[c][t], hid_d[c, t])

                # accumulate g-major across 4 PSUM banks per slice so
                # compute gates on individual 0.5/1MB pieces
                for sl in range(4):
                    vps4 = [vps_pool.tile([P, VC], F32, tag="v",
                                          name=f"vps{i}")
                            for i in range(4)]
                    for g in range(8):
                        for ht in range(4 * g, 4 * g + 4):
                            for st in range(4 * sl, 4 * sl + 4):
                                nc.tensor.matmul(
                                    vps4[st % 4], hid_st(st, ht),
                                    wtv_p[g][:, ht % 4, :],
                                    start=(ht == 0), stop=(ht == HT - 1),
                                )
                    for st in range(4 * sl, 4 * sl + 4):
                        nc.vector.tensor_copy(vkeep[:, st, :], vps4[st % 4])

            # ---------------- phase B+C: Q/K proj + RoPE + attention ------
            trig = ab.enter_context(tc.tile_pool(name="trig", bufs=1))
            cosT = trig.tile([D, S], BF16)
            sinT = trig.tile([D, S], BF16)
            nc.scalar.dma_start(cosT, cos_d[:, :])
            nc.scalar.dma_start(sinT, sin_d[:, :])
            wqk_pool = ab.enter_context(tc.tile_pool(name="wqk", bufs=2))
            rstage = ab.enter_context(tc.tile_pool(name="rstage", bufs=2))
            qkps_pool = ab.enter_context(
                tc.tile_pool(name="qkpsum", bufs=1, space="PSUM"))
            rps_pool = ab.enter_context(
                tc.tile_pool(name="ropepsum", bufs=1, space="PSUM"))

            qk = None
            for pt in range(2 * NH_LOC):  # q0,k0,q1,k1,...
                h, parity = pt // 2, pt % 2
                wqk = wqk_pool.tile([P, HT, P], BF16, tag="w")
                for i in range(4):
                    nc.sync.dma_start(wqk[:, 8 * i:8 * (i + 1), :],
                                      wqk_d[pt][:, 8 * i:8 * (i + 1), :])
                if parity == 0:
                    qk = qk_keep.tile([P, 2, S], BF16, tag="qk")
                for slp in range(2):
                    qk_ps = [qkps_pool.tile([P, SL], F32, tag=f"qk{u}",
                                            name=f"qkps{u}")
                             for u in range(2)]
                    for ht in range(HT):
                        for u in range(2):
                            nc.tensor.matmul(
                                qk_ps[u], wqk[:, ht, :],
                                hid_mv(slp * 2 + u, ht),
                                start=(ht == 0), stop=(ht == HT - 1),
                            )
                        pump(1)
                    for u in range(2):
                        sl = slp * 2 + u
                        qt_b = rstage.tile([P, SL], BF16, tag="qt")
                        nc.scalar.copy(qt_b, qk_ps[u])
                        pump(1)
                        rps = rps_pool.tile([P, SL], F32, tag="r")
                        nc.tensor.matmul(rps, p_swap_b, qt_b,
                                         start=True, stop=True)
                        rps_b = rstage.tile([P, SL], BF16, tag="rb")
                        nc.scalar.copy(rps_b, rps)
                        pump(1)
                        t1 = rstage.tile([P, SL], BF16, tag="t1")
                        nc.vector.tensor_tensor(
                            t1, qt_b, cosT[:, sl * SL:(sl + 1) * SL],
                            AluOpType.mult)
                        t2 = rstage.tile([P, SL], BF16, tag="t2")
                        nc.vector.tensor_tensor(
                            t2, rps_b, sinT[:, sl * SL:(sl + 1) * SL],
                            AluOpType.mult)
                        nc.vector.tensor_tensor(
                            qk[:, parity, sl * SL:(sl + 1) * SL], t1, t2,
                            AluOpType.add)
                    if parity == 1:
                        attn_block(h, qk, 2 * slp)
                        attn_block(h, qk, 2 * slp + 1)

        # ---------------- phase E: o_proj (seq-sharded) ----------------
        # Per 512-col chunk, k-tiles t<24 (heads 0-2) accumulate first and
        # drain to SBUF partials; head 3's attention steps pump between
        # these matmuls, so its AllToAll overlaps the pre-phase.  After
        # recv3 lands, the t>=24 remainder accumulates and is added to the
        # partials on the DVE.
        with ExitStack() as e:
            atl_pool = e.enter_context(tc.tile_pool(name="atTl", bufs=1))
            wo_poolA = e.enter_context(tc.tile_pool(name="woA", bufs=1))
            wo_poolB = e.enter_context(tc.tile_pool(name="woB", bufs=1))
            wo_post_pool = e.enter_context(tc.tile_pool(name="wop", bufs=8))
            part_pool = e.enter_context(tc.tile_pool(name="part", bufs=1))
            ops_pool = e.enter_context(
                tc.tile_pool(name="opsum", bufs=1, space="PSUM"))
            ostage = e.enter_context(tc.tile_pool(name="ostage", bufs=2))

            for hh in (1, 2, 3):
                attnT_h[hh] = atl_pool.tile([P, 8, SC], BF16,
                                            name=f"attnT{hh}")

            def load_wo_pre(oc2):
                # a recycled slot must be written by ONE ring only, or
                # cross-ring WAR waits can interlock; chunks alternate
                # between a sync-fed pool and a gpsimd-fed pool
                pool, eng = ((wo_poolA, nc.sync) if oc2 % 2 == 1
                             else (wo_poolB, nc.gpsimd))
                t = pool.tile([P, 24, SL], BF16, tag="wo")
                for i in range(12):
                    eng.dma_start(
                        t[:, 2 * i:2 * (i + 1), :],
                        wopre_d[oc2][:, 2 * i:2 * (i + 1), :])
                return t

            def load_wo_post(oc2):
                t = wo_post_pool.tile([P, 8, SL], BF16, tag="wp")
                for i in range(2):
                    nc.scalar.dma_start(t[:, 4 * i:4 * (i + 1), :],
                                        wopost_d[oc2][:, 4 * i:4 * (i + 1), :])
                return t

            def atT(t, st_):  # stationary [P, P]: global k-tile t, half st_
                return attnT_h[t // 8][:, t % 8, st_ * P:(st_ + 1) * P]

            # h1/h2 attnT on gpsimd (their collectives are long done;
            # the scalar ring stays clear for the pumped exp chain)
            load_attnT(1, nc.gpsimd)
            load_attnT(2, nc.gpsimd)
            wo_pre = [load_wo_pre(0), load_wo_pre(1)]
            wo_post = []

            # drain a large slice of head 3's attention backlog first: pure
            # PE work that covers chunk 0's weight-stream latency and fires
            # the last AllToAll as early as possible
            pump(16)

            parts = {}
            for c in range(8):
                ops = [ops_pool.tile([P, SL], F32, tag=f"o{c % 2}_{s}",
                                     name=f"ops{c % 2}_{s}")
                       for s in range(2)]
                for t in range(24):
                    for st_ in range(2):
                        nc.tensor.matmul(
                            ops[st_], atT(t, st_), wo_pre[c][:, t, :],
                            start=(t == 0), stop=(t == 23),
                        )
                    pump(2)
                if c + 2 < 8:
                    wo_pre.append(load_wo_pre(c + 2))
                if c < 2:
                    wo_post.extend(load_wo_post(2 * c + i) for i in range(2))
                elif c + 2 < 8:
                    wo_post.append(load_wo_post(c + 2))
                for st_ in range(2):
                    pb = part_pool.tile([P, SL], F32, tag=f"p{c}_{st_}",
                                        name=f"part{c}_{st_}")
                    nc.scalar.copy(pb, ops[st_])
                    parts[(c, st_)] = pb
            pump_all()
            # post-phase: t >= 24 (gated by the last AllToAll); weights are
            # already resident, so this is pure PE work
            for c in range(8):
                ops = [ops_pool.tile([P, SL], F32, tag=f"o{c % 2}_{s}",
                                     name=f"ops{c % 2}_{s}")
                       for s in range(2)]
                for t in range(24, HT):
                    for st_ in range(2):
                        nc.tensor.matmul(
                            ops[st_], atT(t, st_), wo_post[c][:, t - 24, :],
                            start=(t == 24), stop=(t == HT - 1),
                        )
                for st_ in range(2):
                    ob = ostage.tile([P, SL], F32, tag="ob")
                    nc.vector.tensor_tensor(
                        ob, parts[(c, st_)], ops[st_], AluOpType.add)
                    nc.scalar.dma_start(
                        out_d[st_ * P:(st_ + 1) * P,
                              c * SL:(c + 1) * SL],
                        ob,
                    )

    nc.compile()
    return nc


def make_in_maps(hidden_states, position_ids, W_pack, W_o):
    bf = ml_dtypes.bfloat16
    hidden = np.asarray(hidden_states, dtype=np.float32).reshape(S, H)
    W_pack = np.asarray(W_pack, dtype=np.float32)
    W_o = np.asarray(W_o, dtype=np.float32)
    pos = np.asarray(position_ids).reshape(S).astype(np.float64)

    # hidden^T tiled per (s-slice, h-tile): [sl][ht][p][512]
    hid_t = np.ascontiguousarray(
        hidden.T.reshape(HT, P, NSL, SL).transpose(2, 0, 1, 3)).astype(bf)

    inv_freq = 1.0 / (10000.0 ** (np.arange(0, D, 2, dtype=np.float64) / D))
    freqs = np.outer(pos, inv_freq)  # [S, D/2]
    emb = np.concatenate([freqs, freqs], axis=1)  # [S, D]
    cos_t = np.ascontiguousarray(np.cos(emb).T).astype(bf)  # [D, S]
    sin_t = np.ascontiguousarray(np.sin(emb).T).astype(bf)

    # W_o^T partition-major per 512-col o-chunk:
    #   wo_pre_t[oc2][p][t=hh*8+g][c] = W_o[oc2*512+c, (g*4+hh)*128+p], hh<3
    #   wo_post_t[oc2][p][g][c]       = same with hh=3
    woT = np.ascontiguousarray(W_o.T).astype(bf)  # [h', o]
    w5 = woT.reshape(N_CORES, NH_LOC, P, 8, SL)  # [g, hh, p, oc2, c]
    wo_pre_t = np.ascontiguousarray(
        w5[:, :3].transpose(3, 2, 1, 0, 4).reshape(8, P, 24, SL))
    wo_post_t = np.ascontiguousarray(
        w5[:, 3].transpose(2, 1, 0, 3).reshape(8, P, 8, SL))

    in_maps = []
    for c in range(N_CORES):
        # wqk_t[pt][p][ht][cc]: pt = 2*hh+parity (q/k of local head hh)
        wqk_t = np.empty((2 * NH_LOC, P, HT, P), dtype=bf)
        for hh in range(NH_LOC):
            q_rows = W_pack[c * VC + hh * P:c * VC + (hh + 1) * P]
            k_rows = W_pack[H + c * VC + hh * P:H + c * VC + (hh + 1) * P]
            for par, rows in ((0, q_rows), (1, k_rows)):
                # rows [128, H] -> [H, 128] -> [ht, p, cc] -> [p, ht, cc]
                wqk_t[2 * hh + par] = np.ascontiguousarray(
                    rows.T.reshape(HT, P, P).transpose(1, 0, 2)).astype(bf)
        # wtv_t[p][ht][c]: v cols for local heads, partition-major
        wtv = W_pack[2 * H + c * VC:2 * H + (c + 1) * VC].T  # [H, VC]
        wtv_t = np.ascontiguousarray(
            wtv.reshape(HT, P, VC).transpose(1, 0, 2)).astype(bf)
        in_maps.append({
            "hid_t": hid_t,
            "wqk_t": wqk_t,
            "wtv_t": wtv_t,
            "wo_pre_t": wo_pre_t,
            "wo_post_t": wo_post_t,
            "cos_t": cos_t,
            "sin_t": sin_t,
        })
    return in_maps


_NC_CACHE = None


def get_nc():
    global _NC_CACHE
    if _NC_CACHE is None:
        _NC_CACHE = build_nc()
    return _NC_CACHE


def run(inputs, trace=False):
    """Run on hardware; returns (output [1,S,H] f32, BassKernelResults)."""
    in_maps = make_in_maps(
        inputs["hidden_states"], inputs["position_ids"],
        inputs["W_pack"], inputs["W_o"])
    nc = get_nc()
    res = run_bass_kernel_spmd(nc, in_maps, list(range(N_CORES)), trace=trace)
    parts = [np.asarray(res.results[c]["out_s"]) for c in range(N_CORES)]
    out = np.concatenate(parts, axis=0)[None]  # [1, S, H]
    return out.astype(np.float32), res


def kernel(**inputs):
    out, _ = run(inputs, trace=False)
    return out


# revision 23
# speedup vs baseline: 1.0334x; 1.0334x over previous
"""Trainium2 Bass kernel for nn_Attention_78280073937702.

Dense transformer attention block (prefill, B=1, S=2048, H=4096, 32 heads,
head_dim=128, fp32) sharded tensor-parallel over heads across 8 NeuronCores
(4 heads per core), with an AllToAll reshard so o_proj is sequence-sharded.

Host side pre-tiles and pre-casts everything to bf16 in partition-major
contiguous layouts, so every DMA moves multi-KB contiguous runs per
partition (the DMA system is packet-count bound, not byte bound):
  hid_t  [32][128][2048]  hidden^T as per-h-tile [p][s] blocks
  wqk_t  [8][128][32][128] W_pack^T q/k cols per pt (q0,k0,q1,k1,...)
  wtv_t  [128][32][512]    W_pack^T v cols, partition-major
  wo_pre_t  [8][128][24][512]  W_o^T rows for heads 0-2, per o-chunk
  wo_post_t [8][128][8][512]   W_o^T rows for head 3, per o-chunk
  cos_t/sin_t [128][2048]  RoPE tables (from position_ids)

Device per core:
  1. V proj -> vkeep in SBUF (no DRAM roundtrip)
  2. per head: Q proj, K proj (PSUM fp32 -> bf16), RoPE (PE half-swap matmul
     + DVE mul/add, all-bf16 operands), kept in SBUF
  3. causal attention per head (S^T tiles -> exp bf16 -> mask -> esum bf16
     tree -> Z via ones-matmul colsum -> U^T = V E in PSUM -> attn^T bf16).
     Attention is emitted as a queue of small steps pumped one at a time
     between projection (and o_proj) matmuls, so the PE never stalls on the
     scalar/vector exp chain and the HAM clock gate never re-throttles.
  4. per head AllToAll resharding attn^T from head-sharded to seq-sharded
  5. o_proj: out[s, o] natural layout, stationary = attnT tiles, moving =
     streamed W_o^T -> out_s [256, 4096] f32.  Head 3's attention steps pump
     into the pre-phase chunks so the last AllToAll overlaps the pre-phase;
     head 3's weights fully preload during the pre-phase so the post-phase
     does no DMA at all.

Host concatenates the 8 out_s slices along s.
"""

import os
import sys
from collections import deque
from contextlib import ExitStack

import numpy as np
import ml_dtypes

for _p in ("/opt/trn_rl_repo", os.path.expanduser("~/.axon_site/_ro/trn_rl_repo")):
    if os.path.isdir(_p) and _p not in sys.path:
        sys.path.insert(0, _p)

import concourse.bacc as bacc  # noqa: E402
import concourse.bass as bass  # noqa: E402
import concourse.mybir as mybir  # noqa: E402
import concourse.tile as tile  # noqa: E402
from concourse.alu_op_type import AluOpType  # noqa: E402
from concourse.bass_utils import run_bass_kernel_spmd  # noqa: E402

F32 = mybir.dt.float32
BF16 = mybir.dt.bfloat16
EXPF = mybir.ActivationFunctionType.Exp

N_CORES = 8
S = 2048
H = 4096
D = 128
P = 128
N_HEADS = 32
NH_LOC = N_HEADS // N_CORES  # 4 heads per core
HT = H // P  # 32 h-tiles
ST = S // P  # 16 s-tiles
SL = 512  # s-slice width for matmul free dim
NSL = S // SL  # 4
VC = NH_LOC * D  # 512 local v columns
SC = S // N_CORES  # 256 seq cols per core after reshard
NORM = 1.0 / float(np.sqrt(D))


def build_nc():
    nc = bacc.Bacc("TRN2", target_bir_lowering=False, num_devices=N_CORES)

    hid_d = nc.dram_tensor("hid_t", [NSL, HT, P, SL], BF16,
                           kind="ExternalInput")
    wqk_d = nc.dram_tensor("wqk_t", [2 * NH_LOC, P, HT, P], BF16,
                           kind="ExternalInput")
    wtv_d = nc.dram_tensor("wtv_t", [P, HT, VC], BF16, kind="ExternalInput")
    wopre_d = nc.dram_tensor("wo_pre_t", [8, P, 24, SL], BF16,
                             kind="ExternalInput")
    wopost_d = nc.dram_tensor("wo_post_t", [8, P, 8, SL], BF16,
                              kind="ExternalInput")
    cos_d = nc.dram_tensor("cos_t", [D, S], BF16, kind="ExternalInput")
    sin_d = nc.dram_tensor("sin_t", [D, S], BF16, kind="ExternalInput")
    out_d = nc.dram_tensor("out_s", [SC, H], F32, kind="ExternalOutput")

    with tile.TileContext(nc) as tc, ExitStack() as ctx:
        dram = ctx.enter_context(tc.tile_pool(name="dram", bufs=1, space="DRAM"))
        attn_loc = [
            dram.tile([N_CORES, P, SC], BF16, name=f"aloc{h}")
            for h in range(NH_LOC)
        ]
        attn_recv = [
            dram.tile([N_CORES, P, SC], BF16, name=f"arecv{h}")
            for h in range(NH_LOC)
        ]

        # ---------------- constants ----------------
        consts = ctx.enter_context(tc.tile_pool(name="consts", bufs=1))
        tri01_b = consts.tile([P, P], BF16)
        ones_b = consts.tile([P, P], BF16)
        p_swap_b = consts.tile([P, P], BF16)
        with tc.tile_pool(name="cscratch", bufs=1) as cs:
            ones_t = cs.tile([P, P], F32)
            nc.gpsimd.memset(ones_t, 1.0)
            # upper-triangular-with-diag keep-mask [k, q]: keep q >= k
            tri01 = cs.tile([P, P], F32)
            nc.gpsimd.affine_select(
                out=tri01, in_=ones_t, compare_op=AluOpType.is_ge,
                fill=0.0, base=0, channel_multiplier=-1, pattern=[[1, P]],
            )
            nc.vector.tensor_copy(tri01_b, tri01)
            nc.vector.tensor_copy(ones_b, ones_t)
            # signed half-swap lhsT: [i, i+64] = +1 (i<64), [i, i-64] = -1
            neg_t = cs.tile([P, P], F32)
            nc.gpsimd.memset(neg_t, -1.0)
            sw_pos = cs.tile([P, P], F32)
            nc.gpsimd.affine_select(
                out=sw_pos, in_=ones_t, compare_op=AluOpType.is_equal,
                fill=0.0, base=-64, channel_multiplier=-1, pattern=[[1, P]],
            )
            sw_neg = cs.tile([P, P], F32)
            nc.gpsimd.affine_select(
                out=sw_neg, in_=neg_t, compare_op=AluOpType.is_equal,
                fill=0.0, base=64, channel_multiplier=-1, pattern=[[1, P]],
            )
            p_swap = cs.tile([P, P], F32)
            nc.vector.tensor_add(p_swap, sw_pos, sw_neg)
            nc.vector.tensor_copy(p_swap_b, p_swap)

        # long-lived SBUF state (created before the hidden-resident pools)
        qk_keep = ctx.enter_context(tc.tile_pool(name="qkkeep", bufs=2))
        vk_pool = ctx.enter_context(tc.tile_pool(name="vkeep", bufs=1))
        e_pool = ctx.enter_context(tc.tile_pool(name="epool", bufs=4))
        z_pool = ctx.enter_context(tc.tile_pool(name="zpool", bufs=2))
        zr_pool = ctx.enter_context(tc.tile_pool(name="zrpool", bufs=1))
        att_pool = ctx.enter_context(tc.tile_pool(name="attst", bufs=1))
        at_pool = ctx.enter_context(tc.tile_pool(name="atT", bufs=1))
        st_ps_pool = ctx.enter_context(
            tc.tile_pool(name="stpsum", bufs=2, space="PSUM"))
        u_ps_pool = ctx.enter_context(
            tc.tile_pool(name="upsum", bufs=2, space="PSUM"))

        # V stays in SBUF: [p(s-within-tile)][st][c] with c = hh*128 + d
        vkeep = vk_pool.tile([P, ST, VC], BF16)
        # o_proj stationary for head 0 (loaded right after its AllToAll);
        # heads 1-3 tiles live in the o_proj scope
        attnT_h = [at_pool.tile([P, 8, SC], BF16, name="attnT0"),
                   None, None, None]

        def load_attnT(hh, eng):
            for g in range(N_CORES):
                eng.dma_start(attnT_h[hh][:, g, :], attn_recv[hh][g])

        def on_collective(h):
            # h0: tile exists from the start; h3: its tile exists by the
            # time this fires (pumped inside the o_proj scope); h1/h2 are
            # loaded at o_proj start instead
            if h in (0, 3):
                load_attnT(h, nc.gpsimd)

        # ---------- attention step queue (pumped between matmuls) ----------
        pending = deque()

        def pump(n=1):
            for _ in range(n):
                if pending:
                    pending.popleft()()

        def pump_all():
            while pending:
                pending.popleft()()

        def attn_block(h, qk, j):
            """Queue the emission steps for attention block (h, j)."""
            nkt = 4 * j + 4
            blk = {}

            def s_step(i):
                def f():
                    r = i - 4 * j
                    off = max(0, r) * P
                    if i == 0:
                        blk["esum"] = z_pool.tile([P, SL], BF16, tag="es",
                                                  name="esum")
                        blk["et"] = []
                    et = e_pool.tile([P, SL], BF16, tag="e", name="et")
                    sp = st_ps_pool.tile([P, SL], F32, tag="st", name="sp")
                    nc.tensor.matmul(
                        sp[:, off:],
                        qk[:, 1, i * P:(i + 1) * P],
                        qk[:, 0, j * SL + off:(j + 1) * SL],
                        start=True, stop=True,
                    )
                    nc.scalar.activation(
                        et[:, off:], sp[:, off:], EXPF, scale=NORM)
                    if r >= 0:
                        nc.vector.tensor_tensor(
                            et[:, off:off + P], et[:, off:off + P],
                            tri01_b, AluOpType.mult)
                    if i == 0:
                        nc.vector.tensor_copy(blk["esum"], et)
                    else:
                        nc.vector.tensor_tensor(
                            blk["esum"][:, off:], blk["esum"][:, off:],
                            et[:, off:], AluOpType.add)
                    blk["et"].append(et)
                return f

            def u_step(ui):
                def f():
                    if ui == 0:
                        blk["u"] = u_ps_pool.tile([P, SL], F32, tag="u",
                                                  name="u")
                    uoff = max(0, ui - 4 * j) * P
                    nc.tensor.matmul(
                        blk["u"][:, uoff:],
                        vkeep[:, ui, h * P:(h + 1) * P],
                        blk["et"][ui][:, uoff:],
                        start=(ui == 0), stop=(ui == nkt - 1),
                    )
                return f

            def flush_step():
                def f():
                    # z colsum shares the u psum slots
                    zb = u_ps_pool.tile([P, SL], F32, tag="u", name="zb")
                    nc.tensor.matmul(zb, ones_b, blk["esum"],
                                     start=True, stop=True)
                    zr = zr_pool.tile([P, SL], F32, tag="zr", name="zr")
                    nc.vector.reciprocal(zr, zb)
                    att = att_pool.tile([P, SL], BF16, tag="a", name="att")
                    nc.vector.tensor_tensor(att, blk["u"], zr, AluOpType.mult)
                    # stores + collective ride the idle gpsimd ring
                    nc.gpsimd.dma_start(attn_loc[h][2 * j], att[:, :SC])
                    nc.gpsimd.dma_start(attn_loc[h][2 * j + 1], att[:, SC:])
                    if j == NSL - 1:
                        nc.gpsimd.collective_compute(
                            "AllToAll", AluOpType.bypass,
                            replica_groups=[list(range(N_CORES))],
                            ins=[attn_loc[h][:].opt()],
                            outs=[attn_recv[h][:].opt()],
                        )
                        on_collective(h)
                return f

            for i in range(nkt):
                pending.append(s_step(i))
                if i >= 3:
                    pending.append(u_step(i - 3))
            for ui in range(nkt - 3, nkt):
                pending.append(u_step(ui))
            pending.append(flush_step())

        with ExitStack() as ab:  # projection phase
            hidT_pool = ab.enter_context(tc.tile_pool(name="hidT", bufs=1))
            # one tile per (s-slice, h-tile): [p][512], each a fully
            # contiguous 0.25MB DMA; many small DMAs keep many HW queues
            # busy (per-queue bandwidth is only ~22GB/s)
            hidT_t = [[hidT_pool.tile([P, SL], BF16, name=f"h{c}t{t}")
                       for t in range(HT)] for c in range(NSL)]

            def hid_mv(sl, ht):  # moving [P, SL] for s-slice sl, h-tile ht
                return hidT_t[sl][ht]

            def hid_st(st, ht):  # stationary [P, P] for s-tile st, h-tile ht
                return hidT_t[st // 4][ht][:, (st % 4) * P:(st % 4 + 1) * P]

            # ---------------- phase A: V projection ----------------
            with ExitStack() as vblk:
                wtv_pool = vblk.enter_context(tc.tile_pool(name="wtv", bufs=1))
                vps_pool = vblk.enter_context(
                    tc.tile_pool(name="vpsum", bufs=4, space="PSUM"))
                wtv_p = [wtv_pool.tile([P, 4, VC], BF16, name=f"wtv{g}")
                         for g in range(8)]
                # interleave weight/activation pieces in consumption
                # order, alternating rings (each engine ring feeds only ~8
                # HW queues at ~22GB/s each, so one ring tops out ~176GB/s)
                rr = [nc.sync, nc.scalar]
                for g in range(8):
                    rr[g % 2].dma_start(wtv_p[g], wtv_d[:, 4 * g:4 * g + 4, :])
                    for t in range(4 * g, 4 * g + 4):
                        rr[(t + 1) % 2].dma_start(hidT_t[0][t], hid_d[0, t])
                for c in range(1, NSL):
                    for t in range(HT):
                        rr[t % 2].dma_start(hidT_t[c][t], hid_d[c, t])

                # accumulate g-major across 4 PSUM banks per slice so
                # compute gates on individual 0.5/1MB pieces
                for sl in range(4):
                    vps4 = [vps_pool.tile([P, VC], F32, tag="v",
                                          name=f"vps{i}")
                            for i in range(4)]
                    for g in range(8):
                        for ht in range(4 * g, 4 * g + 4):
                            for st in range(4 * sl, 4 * sl + 4):
                                nc.tensor.matmul(
                                    vps4[st % 4], hid_st(st, ht),
                                    wtv_p[g][:, ht % 4, :],
                                    start=(ht == 0), stop=(ht == HT - 1),
                                )
                    for st in range(4 * sl, 4 * sl + 4):
                        nc.vector.tensor_copy(vkeep[:, st, :], vps4[st % 4])

            # ---------------- phase B+C: Q/K proj + RoPE + attention ------
            trig = ab.enter_context(tc.tile_pool(name="trig", bufs=1))
            cosT = trig.tile([D, S], BF16)
            sinT = trig.tile([D, S], BF16)
            nc.scalar.dma_start(cosT, cos_d[:, :])
            nc.scalar.dma_start(sinT, sin_d[:, :])
            wqk_pool = ab.enter_context(tc.tile_pool(name="wqk", bufs=2))
            rstage = ab.enter_context(tc.tile_pool(name="rstage", bufs=2))
            qkps_pool = ab.enter_context(
                tc.tile_pool(name="qkpsum", bufs=1, space="PSUM"))
            rps_pool = ab.enter_context(
                tc.tile_pool(name="ropepsum", bufs=1, space="PSUM"))

            qk = None
            for pt in range(2 * NH_LOC):  # q0,k0,q1,k1,...
                h, parity = pt // 2, pt % 2
                wqk = wqk_pool.tile([P, HT, P], BF16, tag="w")
                for i in range(4):
                    nc.sync.dma_start(wqk[:, 8 * i:8 * (i + 1), :],
                                      wqk_d[pt][:, 8 * i:8 * (i + 1), :])
                if parity == 0:
                    qk = qk_keep.tile([P, 2, S], BF16, tag="qk")
                for slp in range(2):
                    qk_ps = [qkps_pool.tile([P, SL], F32, tag=f"qk{u}",
                                            name=f"qkps{u}")
                             for u in range(2)]
                    for ht in range(HT):
                        for u in range(2):
                            nc.tensor.matmul(
                                qk_ps[u], wqk[:, ht, :],
                                hid_mv(slp * 2 + u, ht),
                                start=(ht == 0), stop=(ht == HT - 1),
                            )
                        pump(1)
                    for u in range(2):
                        sl = slp * 2 + u
                        qt_b = rstage.tile([P, SL], BF16, tag="qt")
                        nc.scalar.copy(qt_b, qk_ps[u])
                        pump(1)
                        rps = rps_pool.tile([P, SL], F32, tag="r")
                        nc.tensor.matmul(rps, p_swap_b, qt_b,
                                         start=True, stop=True)
                        rps_b = rstage.tile([P, SL], BF16, tag="rb")
                        nc.scalar.copy(rps_b, rps)
                        pump(1)
                        t1 = rstage.tile([P, SL], BF16, tag="t1")
                        nc.vector.tensor_tensor(
                            t1, qt_b, cosT[:, sl * SL:(sl + 1) * SL],
                            AluOpType.mult)
                        t2 = rstage.tile([P, SL], BF16, tag="t2")
                        nc.vector.tensor_tensor(
                            t2, rps_b, sinT[:, sl * SL:(sl + 1) * SL],
                            AluOpType.mult)
                        nc.vector.tensor_tensor(
                            qk[:, parity, sl * SL:(sl + 1) * SL], t1, t2,
                            AluOpType.add)
                    if parity == 1:
                        attn_block(h, qk, 2 * slp)
                        attn_block(h, qk, 2 * slp + 1)

        # ---------------- phase E: o_proj (seq-sharded) ----------------
        # Per 512-col chunk, k-tiles t<24 (heads 0-2) accumulate first and
        # drain to SBUF partials; head 3's attention steps pump between
        # these matmuls, so its AllToAll overlaps the pre-phase.  After
        # recv3 lands, the t>=24 remainder accumulates and is added to the
        # partials on the DVE.
        with ExitStack() as e:
            atl_pool = e.enter_context(tc.tile_pool(name="atTl", bufs=1))
            wo_poolA = e.enter_context(tc.tile_pool(name="woA", bufs=1))
            wo_poolB = e.enter_context(tc.tile_pool(name="woB", bufs=1))
            wo_post_pool = e.enter_context(tc.tile_pool(name="wop", bufs=8))
            part_pool = e.enter_context(tc.tile_pool(name="part", bufs=1))
            ops_pool = e.enter_context(
                tc.tile_pool(name="opsum", bufs=1, space="PSUM"))
            ostage = e.enter_context(tc.tile_pool(name="ostage", bufs=2))

            for hh in (1, 2, 3):
                attnT_h[hh] = atl_pool.tile([P, 8, SC], BF16,
                                            name=f"attnT{hh}")

            def load_wo_pre(oc2):
                # a recycled slot must be written by ONE ring only, or
                # cross-ring WAR waits can interlock; chunks alternate
                # between a sync-fed pool and a gpsimd-fed pool
                pool, eng = ((wo_poolA, nc.sync) if oc2 % 2 == 0
                             else (wo_poolB, nc.gpsimd))
                t = pool.tile([P, 24, SL], BF16, tag="wo")
                for i in range(12):
                    eng.dma_start(
                        t[:, 2 * i:2 * (i + 1), :],
                        wopre_d[oc2][:, 2 * i:2 * (i + 1), :])
                return t

            def load_wo_post(oc2):
                t = wo_post_pool.tile([P, 8, SL], BF16, tag="wp")
                for i in range(2):
                    nc.scalar.dma_start(t[:, 4 * i:4 * (i + 1), :],
                                        wopost_d[oc2][:, 4 * i:4 * (i + 1), :])
                return t

            def atT(t, st_):  # stationary [P, P]: global k-tile t, half st_
                return attnT_h[t // 8][:, t % 8, st_ * P:(st_ + 1) * P]

            # h1/h2 attnT on gpsimd (their collectives are long done;
            # the scalar ring stays clear for the pumped exp chain)
            load_attnT(1, nc.gpsimd)
            load_attnT(2, nc.gpsimd)
            wo_pre = [load_wo_pre(0), load_wo_pre(1)]
            wo_post = []

            # drain a large slice of head 3's attention backlog first: pure
            # PE work that covers chunk 0's weight-stream latency and fires
            # the last AllToAll as early as possible
            pump(16)

            parts = {}
            for c in range(8):
                ops = [ops_pool.tile([P, SL], F32, tag=f"o{c % 2}_{s}",
                                     name=f"ops{c % 2}_{s}")
                       for s in range(2)]
                for t in range(24):
                    for st_ in range(2):
                        nc.tensor.matmul(
                            ops[st_], atT(t, st_), wo_pre[c][:, t, :],
                            start=(t == 0), stop=(t == 23),
                        )
                    pump(2)
                if c + 2 < 8:
                    wo_pre.append(load_wo_pre(c + 2))
                if c < 2:
                    wo_post.extend(load_wo_post(2 * c + i) for i in range(2))
                elif c + 2 < 8:
                    wo_post.append(load_wo_post(c + 2))
                for st_ in range(2):
                    pb = part_pool.tile([P, SL], F32, tag=f"p{c}_{st_}",
                                        name=f"part{c}_{st_}")
                    nc.scalar.copy(pb, ops[st_])
                    parts[(c, st_)] = pb
            pump_all()
            # post-phase: t >= 24 (gated by the last AllToAll); weights are
            # already resident, so this is pure PE work
            for c in range(8):
                ops = [ops_pool.tile([P, SL], F32, tag=f"o{c % 2}_{s}",
                                     name=f"ops{c % 2}_{s}")
                       for s in range(2)]
                for t in range(24, HT):
                    for st_ in range(2):
                        nc.tensor.matmul(
                            ops[st_], atT(t, st_), wo_post[c][:, t - 24, :],
                            start=(t == 24), stop=(t == HT - 1),
                        )
                for st_ in range(2):
                    ob = ostage.tile([P, SL], F32, tag="ob")
                    nc.vector.tensor_tensor(
                        ob, parts[(c, st_)], ops[st_], AluOpType.add)
                    nc.scalar.dma_start(
                        out_d[st_ * P:(st_ + 1) * P,
                              c * SL:(c + 1) * SL],
                        ob,
                    )

    nc.compile()
    return nc


def make_in_maps(hidden_states, position_ids, W_pack, W_o):
    bf = ml_dtypes.bfloat16
    hidden = np.asarray(hidden_states, dtype=np.float32).reshape(S, H)
    W_pack = np.asarray(W_pack, dtype=np.float32)
    W_o = np.asarray(W_o, dtype=np.float32)
    pos = np.asarray(position_ids).reshape(S).astype(np.float64)

    # hidden^T tiled per (s-slice, h-tile): [sl][ht][p][512]
    hid_t = np.ascontiguousarray(
        hidden.T.reshape(HT, P, NSL, SL).transpose(2, 0, 1, 3)).astype(bf)

    inv_freq = 1.0 / (10000.0 ** (np.arange(0, D, 2, dtype=np.float64) / D))
    freqs = np.outer(pos, inv_freq)  # [S, D/2]
    emb = np.concatenate([freqs, freqs], axis=1)  # [S, D]
    cos_t = np.ascontiguousarray(np.cos(emb).T).astype(bf)  # [D, S]
    sin_t = np.ascontiguousarray(np.sin(emb).T).astype(bf)

    # W_o^T partition-major per 512-col o-chunk:
    #   wo_pre_t[oc2][p][t=hh*8+g][c] = W_o[oc2*512+c, (g*4+hh)*128+p], hh<3
    #   wo_post_t[oc2][p][g][c]       = same with hh=3
    woT = np.ascontiguousarray(W_o.T).astype(bf)  # [h', o]
    w5 = woT.reshape(N_CORES, NH_LOC, P, 8, SL)  # [g, hh, p, oc2, c]
    wo_pre_t = np.ascontiguousarray(
        w5[:, :3].transpose(3, 2, 1, 0, 4).reshape(8, P, 24, SL))
    wo_post_t = np.ascontiguousarray(
        w5[:, 3].transpose(2, 1, 0, 3).reshape(8, P, 8, SL))

    in_maps = []
    for c in range(N_CORES):
        # wqk_t[pt][p][ht][cc]: pt = 2*hh+parity (q/k of local head hh)
        wqk_t = np.empty((2 * NH_LOC, P, HT, P), dtype=bf)
        for hh in range(NH_LOC):
            q_rows = W_pack[c * VC + hh * P:c * VC + (hh + 1) * P]
            k_rows = W_pack[H + c * VC + hh * P:H + c * VC + (hh + 1) * P]
            for par, rows in ((0, q_rows), (1, k_rows)):
                # rows [128, H] -> [H, 128] -> [ht, p, cc] -> [p, ht, cc]
                wqk_t[2 * hh + par] = np.ascontiguousarray(
                    rows.T.reshape(HT, P, P).transpose(1, 0, 2)).astype(bf)
        # wtv_t[p][ht][c]: v cols for local heads, partition-major
        wtv = W_pack[2 * H + c * VC:2 * H + (c + 1) * VC].T  # [H, VC]
        wtv_t = np.ascontiguousarray(
            wtv.reshape(HT, P, VC).transpose(1, 0, 2)).astype(bf)
        in_maps.append({
            "hid_t": hid_t,
            "wqk_t": wqk_t,
            "wtv_t": wtv_t,
            "wo_pre_t": wo_pre_t,
            "wo_post_t": wo_post_t,
            "cos_t": cos_t,
            "sin_t": sin_t,
        })
    return in_maps


_NC_CACHE = None


def get_nc():
    global _NC_CACHE
    if _NC_CACHE is None:
        _NC_CACHE = build_nc()
    return _NC_CACHE


def run(inputs, trace=False):
    """Run on hardware; returns (output [1,S,H] f32, BassKernelResults)."""
    in_maps = make_in_maps(
        inputs["hidden_states"], inputs["position_ids"],
        inputs["W_pack"], inputs["W_o"])
    nc = get_nc()
    res = run_bass_kernel_spmd(nc, in_maps, list(range(N_CORES)), trace=trace)
    parts = [np.asarray(res.results[c]["out_s"]) for c in range(N_CORES)]
    out = np.concatenate(parts, axis=0)[None]  # [1, S, H]
    return out.astype(np.float32), res


def kernel(**inputs):
    out, _ = run(inputs, trace=False)
    return out


# revision 24
# speedup vs baseline: 1.0353x; 1.0018x over previous
"""Trainium2 Bass kernel for nn_Attention_78280073937702.

Dense transformer attention block (prefill, B=1, S=2048, H=4096, 32 heads,
head_dim=128, fp32) sharded tensor-parallel over heads across 8 NeuronCores
(4 heads per core), with an AllToAll reshard so o_proj is sequence-sharded.

Host side pre-tiles and pre-casts everything to bf16 in partition-major
contiguous layouts, so every DMA moves multi-KB contiguous runs per
partition (the DMA system is packet-count bound, not byte bound):
  hid_t  [32][128][2048]  hidden^T as per-h-tile [p][s] blocks
  wqk_t  [8][128][32][128] W_pack^T q/k cols per pt (q0,k0,q1,k1,...)
  wtv_t  [128][32][512]    W_pack^T v cols, partition-major
  wo_pre_t  [8][128][24][512]  W_o^T rows for heads 0-2, per o-chunk
  wo_post_t [8][128][8][512]   W_o^T rows for head 3, per o-chunk
  cos_t/sin_t [128][2048]  RoPE tables (from position_ids)

Device per core:
  1. V proj -> vkeep in SBUF (no DRAM roundtrip)
  2. per head: Q proj, K proj (PSUM fp32 -> bf16), RoPE (PE half-swap matmul
     + DVE mul/add, all-bf16 operands), kept in SBUF
  3. causal attention per head (S^T tiles -> exp bf16 -> mask -> esum bf16
     tree -> Z via ones-matmul colsum -> U^T = V E in PSUM -> attn^T bf16).
     Attention is emitted as a queue of small steps pumped one at a time
     between projection (and o_proj) matmuls, so the PE never stalls on the
     scalar/vector exp chain and the HAM clock gate never re-throttles.
  4. per head AllToAll resharding attn^T from head-sharded to seq-sharded
  5. o_proj: out[s, o] natural layout, stationary = attnT tiles, moving =
     streamed W_o^T -> out_s [256, 4096] f32.  Head 3's attention steps pump
     into the pre-phase chunks so the last AllToAll overlaps the pre-phase;
     head 3's weights fully preload during the pre-phase so the post-phase
     does no DMA at all.

Host concatenates the 8 out_s slices along s.
"""

import os
import sys
from collections import deque
from contextlib import ExitStack

import numpy as np
import ml_dtypes

for _p in ("/opt/trn_rl_repo", os.path.expanduser("~/.axon_site/_ro/trn_rl_repo")):
    if os.path.isdir(_p) and _p not in sys.path:
        sys.path.insert(0, _p)

import concourse.bacc as bacc  # noqa: E402
import concourse.bass as bass  # noqa: E402
import concourse.mybir as mybir  # noqa: E402
import concourse.tile as tile  # noqa: E402
from concourse.alu_op_type import AluOpType  # noqa: E402
from concourse.bass_utils import run_bass_kernel_spmd  # noqa: E402

F32 = mybir.dt.float32
BF16 = mybir.dt.bfloat16
EXPF = mybir.ActivationFunctionType.Exp

N_CORES = 8
S = 2048
H = 4096
D = 128
P = 128
N_HEADS = 32
NH_LOC = N_HEADS // N_CORES  # 4 heads per core
HT = H // P  # 32 h-tiles
ST = S // P  # 16 s-tiles
SL = 512  # s-slice width for matmul free dim
NSL = S // SL  # 4
VC = NH_LOC * D  # 512 local v columns
SC = S // N_CORES  # 256 seq cols per core after reshard
NORM = 1.0 / float(np.sqrt(D))


def build_nc():
    nc = bacc.Bacc("TRN2", target_bir_lowering=False, num_devices=N_CORES)

    hid_d = nc.dram_tensor("hid_t", [NSL, HT, P, SL], BF16,
                           kind="ExternalInput")
    wqk_d = nc.dram_tensor("wqk_t", [2 * NH_LOC, P, HT, P], BF16,
                           kind="ExternalInput")
    wtv_d = nc.dram_tensor("wtv_t", [P, HT, VC], BF16, kind="ExternalInput")
    wopre_d = nc.dram_tensor("wo_pre_t", [8, P, 24, SL], BF16,
                             kind="ExternalInput")
    wopost_d = nc.dram_tensor("wo_post_t", [8, P, 8, SL], BF16,
                              kind="ExternalInput")
    cos_d = nc.dram_tensor("cos_t", [D, S], BF16, kind="ExternalInput")
    sin_d = nc.dram_tensor("sin_t", [D, S], BF16, kind="ExternalInput")
    out_d = nc.dram_tensor("out_s", [SC, H], F32, kind="ExternalOutput")

    with tile.TileContext(nc) as tc, ExitStack() as ctx:
        dram = ctx.enter_context(tc.tile_pool(name="dram", bufs=1, space="DRAM"))
        attn_loc = [
            dram.tile([N_CORES, P, SC], BF16, name=f"aloc{h}")
            for h in range(NH_LOC)
        ]
        attn_recv = [
            dram.tile([N_CORES, P, SC], BF16, name=f"arecv{h}")
            for h in range(NH_LOC)
        ]

        # ---------------- constants ----------------
        consts = ctx.enter_context(tc.tile_pool(name="consts", bufs=1))
        tri01_b = consts.tile([P, P], BF16)
        ones_b = consts.tile([P, P], BF16)
        p_swap_b = consts.tile([P, P], BF16)
        with tc.tile_pool(name="cscratch", bufs=1) as cs:
            ones_t = cs.tile([P, P], F32)
            nc.gpsimd.memset(ones_t, 1.0)
            # upper-triangular-with-diag keep-mask [k, q]: keep q >= k
            tri01 = cs.tile([P, P], F32)
            nc.gpsimd.affine_select(
                out=tri01, in_=ones_t, compare_op=AluOpType.is_ge,
                fill=0.0, base=0, channel_multiplier=-1, pattern=[[1, P]],
            )
            nc.vector.tensor_copy(tri01_b, tri01)
            nc.vector.tensor_copy(ones_b, ones_t)
            # signed half-swap lhsT: [i, i+64] = +1 (i<64), [i, i-64] = -1
            neg_t = cs.tile([P, P], F32)
            nc.gpsimd.memset(neg_t, -1.0)
            sw_pos = cs.tile([P, P], F32)
            nc.gpsimd.affine_select(
                out=sw_pos, in_=ones_t, compare_op=AluOpType.is_equal,
                fill=0.0, base=-64, channel_multiplier=-1, pattern=[[1, P]],
            )
            sw_neg = cs.tile([P, P], F32)
            nc.gpsimd.affine_select(
                out=sw_neg, in_=neg_t, compare_op=AluOpType.is_equal,
                fill=0.0, base=64, channel_multiplier=-1, pattern=[[1, P]],
            )
            p_swap = cs.tile([P, P], F32)
            nc.vector.tensor_add(p_swap, sw_pos, sw_neg)
            nc.vector.tensor_copy(p_swap_b, p_swap)

        # long-lived SBUF state (created before the hidden-resident pools)
        qk_keep = ctx.enter_context(tc.tile_pool(name="qkkeep", bufs=2))
        vk_pool = ctx.enter_context(tc.tile_pool(name="vkeep", bufs=1))
        e_pool = ctx.enter_context(tc.tile_pool(name="epool", bufs=4))
        z_pool = ctx.enter_context(tc.tile_pool(name="zpool", bufs=2))
        zr_pool = ctx.enter_context(tc.tile_pool(name="zrpool", bufs=1))
        att_pool = ctx.enter_context(tc.tile_pool(name="attst", bufs=1))
        at_pool = ctx.enter_context(tc.tile_pool(name="atT", bufs=1))
        st_ps_pool = ctx.enter_context(
            tc.tile_pool(name="stpsum", bufs=2, space="PSUM"))
        u_ps_pool = ctx.enter_context(
            tc.tile_pool(name="upsum", bufs=2, space="PSUM"))

        # V stays in SBUF: [p(s-within-tile)][st][c] with c = hh*128 + d
        vkeep = vk_pool.tile([P, ST, VC], BF16)
        # o_proj stationary for head 0 (loaded right after its AllToAll);
        # heads 1-3 tiles live in the o_proj scope
        attnT_h = [at_pool.tile([P, 8, SC], BF16, name="attnT0"),
                   None, None, None]

        def load_attnT(hh, eng):
            for g in range(N_CORES):
                eng.dma_start(attnT_h[hh][:, g, :], attn_recv[hh][g])

        def on_collective(h):
            # h0: tile exists from the start; h3: its tile exists by the
            # time this fires (pumped inside the o_proj scope); h1/h2 are
            # loaded at o_proj start instead
            if h in (0, 3):
                load_attnT(h, nc.gpsimd)

        # ---------- attention step queue (pumped between matmuls) ----------
        pending = deque()

        def pump(n=1):
            for _ in range(n):
                if pending:
                    pending.popleft()()

        def pump_all():
            while pending:
                pending.popleft()()

        def attn_block(h, qk, j):
            """Queue the emission steps for attention block (h, j)."""
            nkt = 4 * j + 4
            blk = {}

            def s_step(i):
                def f():
                    r = i - 4 * j
                    off = max(0, r) * P
                    if i == 0:
                        blk["esum"] = z_pool.tile([P, SL], BF16, tag="es",
                                                  name="esum")
                        blk["et"] = []
                    et = e_pool.tile([P, SL], BF16, tag="e", name="et")
                    sp = st_ps_pool.tile([P, SL], F32, tag="st", name="sp")
                    nc.tensor.matmul(
                        sp[:, off:],
                        qk[:, 1, i * P:(i + 1) * P],
                        qk[:, 0, j * SL + off:(j + 1) * SL],
                        start=True, stop=True,
                    )
                    nc.scalar.activation(
                        et[:, off:], sp[:, off:], EXPF, scale=NORM)
                    if r >= 0:
                        nc.vector.tensor_tensor(
                            et[:, off:off + P], et[:, off:off + P],
                            tri01_b, AluOpType.mult)
                    if i == 0:
                        nc.vector.tensor_copy(blk["esum"], et)
                    else:
                        nc.vector.tensor_tensor(
                            blk["esum"][:, off:], blk["esum"][:, off:],
                            et[:, off:], AluOpType.add)
                    blk["et"].append(et)
                return f

            def u_step(ui):
                def f():
                    if ui == 0:
                        blk["u"] = u_ps_pool.tile([P, SL], F32, tag="u",
                                                  name="u")
                    uoff = max(0, ui - 4 * j) * P
                    nc.tensor.matmul(
                        blk["u"][:, uoff:],
                        vkeep[:, ui, h * P:(h + 1) * P],
                        blk["et"][ui][:, uoff:],
                        start=(ui == 0), stop=(ui == nkt - 1),
                    )
                return f

            def flush_step():
                def f():
                    # z colsum shares the u psum slots
                    zb = u_ps_pool.tile([P, SL], F32, tag="u", name="zb")
                    nc.tensor.matmul(zb, ones_b, blk["esum"],
                                     start=True, stop=True)
                    zr = zr_pool.tile([P, SL], F32, tag="zr", name="zr")
                    nc.vector.reciprocal(zr, zb)
                    att = att_pool.tile([P, SL], BF16, tag="a", name="att")
                    nc.vector.tensor_tensor(att, blk["u"], zr, AluOpType.mult)
                    # stores + collective ride the idle gpsimd ring
                    nc.gpsimd.dma_start(attn_loc[h][2 * j], att[:, :SC])
                    nc.gpsimd.dma_start(attn_loc[h][2 * j + 1], att[:, SC:])
                    if j == NSL - 1:
                        nc.gpsimd.collective_compute(
                            "AllToAll", AluOpType.bypass,
                            replica_groups=[list(range(N_CORES))],
                            ins=[attn_loc[h][:].opt()],
                            outs=[attn_recv[h][:].opt()],
                        )
                        on_collective(h)
                return f

            for i in range(nkt):
                pending.append(s_step(i))
                if i >= 3:
                    pending.append(u_step(i - 3))
            for ui in range(nkt - 3, nkt):
                pending.append(u_step(ui))
            pending.append(flush_step())

        with ExitStack() as ab:  # projection phase
            hidT_pool = ab.enter_context(tc.tile_pool(name="hidT", bufs=1))
            # one tile per (s-slice, h-tile): [p][512], each a fully
            # contiguous 0.25MB DMA; many small DMAs keep many HW queues
            # busy (per-queue bandwidth is only ~22GB/s)
            hidT_t = [[hidT_pool.tile([P, SL], BF16, name=f"h{c}t{t}")
                       for t in range(HT)] for c in range(NSL)]

            def hid_mv(sl, ht):  # moving [P, SL] for s-slice sl, h-tile ht
                return hidT_t[sl][ht]

            def hid_st(st, ht):  # stationary [P, P] for s-tile st, h-tile ht
                return hidT_t[st // 4][ht][:, (st % 4) * P:(st % 4 + 1) * P]

            # ---------------- phase A: V projection ----------------
            with ExitStack() as vblk:
                wtv_pool = vblk.enter_context(tc.tile_pool(name="wtv", bufs=1))
                vps_pool = vblk.enter_context(
                    tc.tile_pool(name="vpsum", bufs=4, space="PSUM"))
                wtv_p = [wtv_pool.tile([P, 4, VC], BF16, name=f"wtv{g}")
                         for g in range(8)]
                # interleave weight/activation pieces in consumption
                # order, alternating rings (each engine ring feeds only ~8
                # HW queues at ~22GB/s each, so one ring tops out ~176GB/s)
                rr = [nc.sync, nc.scalar]
                for g in range(8):
                    rr[g % 2].dma_start(wtv_p[g], wtv_d[:, 4 * g:4 * g + 4, :])
                    for t in range(4 * g, 4 * g + 4):
                        rr[(t + 1) % 2].dma_start(hidT_t[0][t], hid_d[0, t])
                for c in range(1, NSL):
                    for t in range(HT):
                        rr[t % 2].dma_start(hidT_t[c][t], hid_d[c, t])

                # accumulate g-major across 4 PSUM banks per slice so
                # compute gates on individual 0.5/1MB pieces
                for sl in range(4):
                    vps4 = [vps_pool.tile([P, VC], F32, tag="v",
                                          name=f"vps{i}")
                            for i in range(4)]
                    for g in range(8):
                        for ht in range(4 * g, 4 * g + 4):
                            for st in range(4 * sl, 4 * sl + 4):
                                nc.tensor.matmul(
                                    vps4[st % 4], hid_st(st, ht),
                                    wtv_p[g][:, ht % 4, :],
                                    start=(ht == 0), stop=(ht == HT - 1),
                                )
                    for st in range(4 * sl, 4 * sl + 4):
                        nc.vector.tensor_copy(vkeep[:, st, :], vps4[st % 4])

            # ---------------- phase B+C: Q/K proj + RoPE + attention ------
            trig = ab.enter_context(tc.tile_pool(name="trig", bufs=1))
            cosT = trig.tile([D, S], BF16)
            sinT = trig.tile([D, S], BF16)
            nc.scalar.dma_start(cosT, cos_d[:, :])
            nc.scalar.dma_start(sinT, sin_d[:, :])
            wqk_pool = ab.enter_context(tc.tile_pool(name="wqk", bufs=2))
            rstage = ab.enter_context(tc.tile_pool(name="rstage", bufs=2))
            qkps_pool = ab.enter_context(
                tc.tile_pool(name="qkpsum", bufs=1, space="PSUM"))
            rps_pool = ab.enter_context(
                tc.tile_pool(name="ropepsum", bufs=1, space="PSUM"))

            qk = None
            for pt in range(2 * NH_LOC):  # q0,k0,q1,k1,...
                h, parity = pt // 2, pt % 2
                wqk = wqk_pool.tile([P, HT, P], BF16, tag="w")
                for i in range(4):
                    nc.sync.dma_start(wqk[:, 8 * i:8 * (i + 1), :],
                                      wqk_d[pt][:, 8 * i:8 * (i + 1), :])
                if parity == 0:
                    qk = qk_keep.tile([P, 2, S], BF16, tag="qk")
                for slp in range(2):
                    qk_ps = [qkps_pool.tile([P, SL], F32, tag=f"qk{u}",
                                            name=f"qkps{u}")
                             for u in range(2)]
                    for ht in range(HT):
                        for u in range(2):
                            nc.tensor.matmul(
                                qk_ps[u], wqk[:, ht, :],
                                hid_mv(slp * 2 + u, ht),
                                start=(ht == 0), stop=(ht == HT - 1),
                            )
                        pump(1)
                    for u in range(2):
                        sl = slp * 2 + u
                        qt_b = rstage.tile([P, SL], BF16, tag="qt")
                        nc.scalar.copy(qt_b, qk_ps[u])
                        pump(1)
                        rps = rps_pool.tile([P, SL], F32, tag="r")
                        nc.tensor.matmul(rps, p_swap_b, qt_b,
                                         start=True, stop=True)
                        rps_b = rstage.tile([P, SL], BF16, tag="rb")
                        nc.scalar.copy(rps_b, rps)
                        pump(1)
                        t1 = rstage.tile([P, SL], BF16, tag="t1")
                        nc.vector.tensor_tensor(
                            t1, qt_b, cosT[:, sl * SL:(sl + 1) * SL],
                            AluOpType.mult)
                        t2 = rstage.tile([P, SL], BF16, tag="t2")
                        nc.vector.tensor_tensor(
                            t2, rps_b, sinT[:, sl * SL:(sl + 1) * SL],
                            AluOpType.mult)
                        nc.vector.tensor_tensor(
                            qk[:, parity, sl * SL:(sl + 1) * SL], t1, t2,
                            AluOpType.add)
                    if parity == 1:
                        attn_block(h, qk, 2 * slp)
                        attn_block(h, qk, 2 * slp + 1)

        # ---------------- phase E: o_proj (seq-sharded) ----------------
        # Per 512-col chunk, k-tiles t<24 (heads 0-2) accumulate first and
        # drain to SBUF partials; head 3's attention steps pump between
        # these matmuls, so its AllToAll overlaps the pre-phase.  After
        # recv3 lands, the t>=24 remainder accumulates and is added to the
        # partials on the DVE.
        with ExitStack() as e:
            atl_pool = e.enter_context(tc.tile_pool(name="atTl", bufs=1))
            wo_poolA = e.enter_context(tc.tile_pool(name="woA", bufs=1))
            wo_poolB = e.enter_context(tc.tile_pool(name="woB", bufs=1))
            wo_post_pool = e.enter_context(tc.tile_pool(name="wop", bufs=8))
            part_pool = e.enter_context(tc.tile_pool(name="part", bufs=1))
            ops_pool = e.enter_context(
                tc.tile_pool(name="opsum", bufs=1, space="PSUM"))
            ostage = e.enter_context(tc.tile_pool(name="ostage", bufs=2))

            for hh in (1, 2, 3):
                attnT_h[hh] = atl_pool.tile([P, 8, SC], BF16,
                                            name=f"attnT{hh}")

            def load_wo_pre(oc2):
                # a recycled slot must be written by ONE ring only, or
                # cross-ring WAR waits can interlock; chunks alternate
                # between a sync-fed pool and a gpsimd-fed pool
                pool, eng = ((wo_poolA, nc.sync) if oc2 % 2 == 0
                             else (wo_poolB, nc.gpsimd))
                t = pool.tile([P, 24, SL], BF16, tag="wo")
                for i in range(12):
                    eng.dma_start(
                        t[:, 2 * i:2 * (i + 1), :],
                        wopre_d[oc2][:, 2 * i:2 * (i + 1), :])
                return t

            def load_wo_post(oc2):
                t = wo_post_pool.tile([P, 8, SL], BF16, tag="wp")
                for i in range(2):
                    nc.scalar.dma_start(t[:, 4 * i:4 * (i + 1), :],
                                        wopost_d[oc2][:, 4 * i:4 * (i + 1), :])
                return t

            def atT(t, st_):  # stationary [P, P]: global k-tile t, half st_
                return attnT_h[t // 8][:, t % 8, st_ * P:(st_ + 1) * P]

            # h1/h2 attnT on gpsimd (their collectives are long done;
            # the scalar ring stays clear for the pumped exp chain)
            load_attnT(1, nc.gpsimd)
            load_attnT(2, nc.gpsimd)
            wo_pre = [load_wo_pre(0), load_wo_pre(1)]
            wo_post = []

            # drain a large slice of head 3's attention backlog first: pure
            # PE work that covers chunk 0's weight-stream latency and fires
            # the last AllToAll as early as possible
            pump(16)

            parts = {}

            def pre_chunk(c):
                ops = [ops_pool.tile([P, SL], F32, tag=f"o{c % 2}_{s}",
                                     name=f"ops{c % 2}_{s}")
                       for s in range(2)]
                for t in range(24):
                    for st_ in range(2):
                        nc.tensor.matmul(
                            ops[st_], atT(t, st_), wo_pre[c][:, t, :],
                            start=(t == 0), stop=(t == 23),
                        )
                    pump(2)
                if c + 2 < 8:
                    wo_pre.append(load_wo_pre(c + 2))
                if c < 2:
                    wo_post.extend(load_wo_post(2 * c + i) for i in range(2))
                elif c + 2 < 8:
                    wo_post.append(load_wo_post(c + 2))
                for st_ in range(2):
                    pb = part_pool.tile([P, SL], F32, tag=f"p{c}_{st_}",
                                        name=f"part{c}_{st_}")
                    nc.scalar.copy(pb, ops[st_])
                    parts[(c, st_)] = pb

            def post_chunk(c):
                # t >= 24 (gated by the last AllToAll); weights resident,
                # so this is pure PE work with a DMA-free window
                ops = [ops_pool.tile([P, SL], F32, tag=f"o{c % 2}_{s}",
                                     name=f"ops{c % 2}_{s}")
                       for s in range(2)]
                for t in range(24, HT):
                    for st_ in range(2):
                        nc.tensor.matmul(
                            ops[st_], atT(t, st_), wo_post[c][:, t - 24, :],
                            start=(t == 24), stop=(t == HT - 1),
                        )
                for st_ in range(2):
                    ob = ostage.tile([P, SL], F32, tag="ob")
                    nc.vector.tensor_tensor(
                        ob, parts[(c, st_)], ops[st_], AluOpType.add)
                    nc.scalar.dma_start(
                        out_d[st_ * P:(st_ + 1) * P,
                              c * SL:(c + 1) * SL],
                        ob,
                    )

            # interleave the DMA-free post chunks into the late pre
            # schedule so pre 6/7's weight streams ride their windows
            for c in range(6):
                pre_chunk(c)
            pump_all()
            for c in range(4):
                post_chunk(c)
            pre_chunk(6)
            post_chunk(4)
            post_chunk(5)
            pre_chunk(7)
            post_chunk(6)
            post_chunk(7)

    nc.compile()
    return nc


def make_in_maps(hidden_states, position_ids, W_pack, W_o):
    bf = ml_dtypes.bfloat16
    hidden = np.asarray(hidden_states, dtype=np.float32).reshape(S, H)
    W_pack = np.asarray(W_pack, dtype=np.float32)
    W_o = np.asarray(W_o, dtype=np.float32)
    pos = np.asarray(position_ids).reshape(S).astype(np.float64)

    # hidden^T tiled per (s-slice, h-tile): [sl][ht][p][512]
    hid_t = np.ascontiguousarray(
        hidden.T.reshape(HT, P, NSL, SL).transpose(2, 0, 1, 3)).astype(bf)

    inv_freq = 1.0 / (10000.0 ** (np.arange(0, D, 2, dtype=np.float64) / D))
    freqs = np.outer(pos, inv_freq)  # [S, D/2]
    emb = np.concatenate([freqs, freqs], axis=1)  # [S, D]
    cos_t = np.ascontiguousarray(np.cos(emb).T).astype(bf)  # [D, S]
    sin_t = np.ascontiguousarray(np.sin(emb).T).astype(bf)

    # W_o^T partition-major per 512-col o-chunk:
    #   wo_pre_t[oc2][p][t=hh*8+g][c] = W_o[oc2*512+c, (g*4+hh)*128+p], hh<3
    #   wo_post_t[oc2][p][g][c]       = same with hh=3
    woT = np.ascontiguousarray(W_o.T).astype(bf)  # [h', o]
    w5 = woT.reshape(N_CORES, NH_LOC, P, 8, SL)  # [g, hh, p, oc2, c]
    wo_pre_t = np.ascontiguousarray(
        w5[:, :3].transpose(3, 2, 1, 0, 4).reshape(8, P, 24, SL))
    wo_post_t = np.ascontiguousarray(
        w5[:, 3].transpose(2, 1, 0, 3).reshape(8, P, 8, SL))

    in_maps = []
    for c in range(N_CORES):
        # wqk_t[pt][p][ht][cc]: pt = 2*hh+parity (q/k of local head hh)
        wqk_t = np.empty((2 * NH_LOC, P, HT, P), dtype=bf)
        for hh in range(NH_LOC):
            q_rows = W_pack[c * VC + hh * P:c * VC + (hh + 1) * P]
            k_rows = W_pack[H + c * VC + hh * P:H + c * VC + (hh + 1) * P]
            for par, rows in ((0, q_rows), (1, k_rows)):
                # rows [128, H] -> [H, 128] -> [ht, p, cc] -> [p, ht, cc]
                wqk_t[2 * hh + par] = np.ascontiguousarray(
                    rows.T.reshape(HT, P, P).transpose(1, 0, 2)).astype(bf)
        # wtv_t[p][ht][c]: v cols for local heads, partition-major
        wtv = W_pack[2 * H + c * VC:2 * H + (c + 1) * VC].T  # [H, VC]
        wtv_t = np.ascontiguousarray(
            wtv.reshape(HT, P, VC).transpose(1, 0, 2)).astype(bf)
        in_maps.append({
            "hid_t": hid_t,
            "wqk_t": wqk_t,
            "wtv_t": wtv_t,
            "wo_pre_t": wo_pre_t,
            "wo_post_t": wo_post_t,
            "cos_t": cos_t,
            "sin_t": sin_t,
        })
    return in_maps


_NC_CACHE = None


def get_nc():
    global _NC_CACHE
    if _NC_CACHE is None:
        _NC_CACHE = build_nc()
    return _NC_CACHE


def run(inputs, trace=False):
    """Run on hardware; returns (output [1,S,H] f32, BassKernelResults)."""
    in_maps = make_in_maps(
        inputs["hidden_states"], inputs["position_ids"],
        inputs["W_pack"], inputs["W_o"])
    nc = get_nc()
    res = run_bass_kernel_spmd(nc, in_maps, list(range(N_CORES)), trace=trace)
    parts = [np.asarray(res.results[c]["out_s"]) for c in range(N_CORES)]
    out = np.concatenate(parts, axis=0)[None]  # [1, S, H]
    return out.astype(np.float32), res


def kernel(**inputs):
    out, _ = run(inputs, trace=False)
    return out
